# revision 23
# baseline (speedup 1.0000x reference)
"""Block-diagonal linear (segment_reduce) Trainium2 kernel.

y[b, o] = sum_k x[b, o*16 + k] * weight[o, k]
x: (8192, 32768) f32, weight: (2048, 16) f32 -> y: (8192, 2048) f32

Sharding: data-parallel over batch across 8 NeuronCores (1024 rows each);
weight replicated. The full weight row (32768 f32 values viewed flat) is
broadcast across all 128 partitions ONCE at startup by the otherwise-idle
TensorE (K=1 ones-column fp32r matmul into PSUM, ACT copy to fp16) into
four persistent [128, 8192] tiles — no per-chunk rebuild, so the x
stream never stalls on weight recycling (the previous version's ~25-45us
chunk-boundary stalls).

Per core the kernel streams x in (128, 8192) tiles six deep; SWDGE casts
f32->fp16 in flight so HBM reads stay f32 while the vector engine runs
in 16-bit packed mode. Per tile: one in-place multiply by the broadcast
weight, then a 16->1 segmented reduction as a binary tree of fp16
tensor-adds telescoping in place, final level accumulating into fp32.

Shapes probed this session and rejected: full-row [128, 32768] transfers
stream ~30% faster in isolation (430 vs 335 GB/s/core: fewer transfers
avoid SDMA engine stretching), but only two 64-KiB slots fit in SBUF
beside the 64-KiB weight, and the per-tile DVE chain (~42-48us under
load) then gates the 39.4us transfer cadence — measured 600-690us, far
worse than this 6-deep pipeline. GPSIMD tensor-op offload overlaps DVE
but stretches 2-4x under SWDGE emission load; not worth the jitter.
"""

import numpy as np

import concourse.bass as bass
import concourse.mybir as mybir
from concourse.bass_utils import run_bass_kernel_spmd
from concourse.tile import TileContext

B = 8192
IN_F = 32768
OUT_F = 2048
BLK = 16
N_CORES = 8
B_LOC = B // N_CORES  # 1024

CCHUNK = 8192               # feature columns per x tile
SEG = CCHUNK // BLK         # outputs per tile (512)
N_CC = IN_F // CCHUNK       # 4
N_BT = B_LOC // 128         # 8

F32 = mybir.dt.float32
F32R = mybir.dt.float32r
F16 = mybir.dt.float16

_NC_CACHE = {}


def _build(legalize=True, **bass_kwargs):
    key = ("nc", legalize, tuple(sorted(bass_kwargs.items())))
    if key in _NC_CACHE:
        return _NC_CACHE[key]
    nc = bass.Bass(**bass_kwargs)
    x = nc.declare_dram_parameter("x", [B_LOC, IN_F], F32, isOutput=False)
    w = nc.declare_dram_parameter("weight", [OUT_F, BLK], F32R, isOutput=False)
    onesr = nc.declare_dram_parameter("onesr", [1, 128], F32R, isOutput=False)
    y = nc.declare_dram_parameter("y", [B_LOC, OUT_F], F32, isOutput=True)

    wf = w[:].rearrange("o k -> (o k)")  # (32768,) flat, f = o*16 + k

    with TileContext(nc) as tc:
        with (
            tc.tile_pool(name="wpool", bufs=1) as wpool,
            tc.tile_pool(name="wrowp", bufs=1) as wrowp,
            tc.tile_pool(name="xpool", bufs=6) as xpool,
            tc.tile_pool(name="ypool", bufs=4) as ypool,
            tc.tile_pool(name="probe", bufs=2) as probepool,
            tc.tile_pool(name="const", bufs=1) as constp,
            tc.tile_pool(name="psb", bufs=2, space="PSUM") as psb,
        ):
            # ones rides SWDGE so the Q7 ring init is absorbed before the
            # first (much larger) x transfer is emitted.
            ones = constp.tile([1, 128], F32R)
            nc.gpsimd.dma_start(out=ones[:], in_=onesr[:])

            # Broadcast the whole weight across all 128 partitions with the
            # PE: wtile[p, f] = wrow[0, f] via a K=1 ones-column fp32r
            # matmul (saves 16 MiB/core of HBM re-reads). All four chunk
            # tiles persist for the rest of the kernel.
            HALF = CCHUNK // 2
            wtiles = []
            for cc in range(N_CC):
                wtile = wpool.tile([128, CCHUNK], F16, name=f"wt{cc}", tag=f"wt{cc}")
                for h in range(2):
                    wrow = wrowp.tile([1, HALF], F32R, name="wr", tag="wr")
                    off = cc * CCHUNK + h * HALF
                    nc.sync.dma_start(out=wrow[:], in_=wf[off : off + HALF])
                    for s in range(HALF // 512):
                        wps = psb.tile([128, 512], F32)
                        nc.tensor.matmul(
                            out=wps[:, :],
                            lhsT=ones[:, 0:128],
                            rhs=wrow[:, s * 512 : (s + 1) * 512],
                            skip_group_check=True,
                        )
                        col = h * HALF + s * 512
                        nc.scalar.copy(out=wtile[:, col : col + 512], in_=wps[:])
                wtiles.append(wtile)

            for cc in range(N_CC):
                # One cheap DVE read of the finished weight tile anchors the
                # wtile dependency on the DVE queue so the per-tile
                # multiplies only need their own x-DMA wait.
                probe = probepool.tile([1, 1], F32, name="pr", tag="pr")
                nc.vector.tensor_copy(out=probe[:], in_=wtiles[cc][0:1, 0:1])
                # The last four bands of the last chunk stream as 4096-col
                # half tiles: the vector engine trails the x stream by
                # roughly one tile's chain, so smaller final tiles halve the
                # post-stream drain.
                if cc == N_CC - 1:
                    work = (
                        [(bt, 0, CCHUNK) for bt in range(4)]
                        + [
                            (bt, h * (CCHUNK // 2), CCHUNK // 2)
                            for bt in (4, 5)
                            for h in range(2)
                        ]
                        + [
                            (bt, q * (CCHUNK // 4), CCHUNK // 4)
                            for bt in (6, 7)
                            for q in range(4)
                        ]
                    )
                else:
                    work = [(bt, 0, CCHUNK) for bt in range(N_BT)]
                for bt, off, cols in work:
                    # SWDGE DMA casts x to fp16 on the way in, so the
                    # multiply runs in the DVE 2x packed mode.
                    seg = cols // BLK
                    xtile = xpool.tile([128, cols], F16, name="xtile", tag="xtile")
                    col0 = cc * CCHUNK + off
                    nc.gpsimd.dma_start(
                        out=xtile[:],
                        in_=x[bt * 128 : (bt + 1) * 128, col0 : col0 + cols],
                    )
                    nc.vector.tensor_mul(
                        out=xtile[:],
                        in0=xtile[:],
                        in1=wtiles[cc][:, off : off + cols],
                    )
                    # Segmented 16 -> 1 reduction as a binary tree that
                    # telescopes in place (each level's writes trail its
                    # reads); the final level accumulates into fp32.
                    p3 = xtile[:].rearrange("p (s k) -> p s k", k=16)
                    l1 = xtile[:, 0 : cols // 2].rearrange("p (s k) -> p s k", k=8)
                    nc.vector.tensor_add(out=l1, in0=p3[:, :, 0:8], in1=p3[:, :, 8:16])
                    l2 = xtile[:, 0 : cols // 4].rearrange("p (s k) -> p s k", k=4)
                    nc.vector.tensor_add(out=l2, in0=l1[:, :, 0:4], in1=l1[:, :, 4:8])
                    l3 = xtile[:, 0 : cols // 8].rearrange("p (s k) -> p s k", k=2)
                    nc.vector.tensor_add(out=l3, in0=l2[:, :, 0:2], in1=l2[:, :, 2:4])
                    ytile = ypool.tile([128, seg], F32, name="ytile", tag="ytile")
                    nc.vector.tensor_add(
                        out=ytile[:], in0=l3[:, :, 0], in1=l3[:, :, 1]
                    )
                    ycol = col0 // BLK
                    nc.sync.dma_start(
                        out=y[bt * 128 : (bt + 1) * 128, ycol : ycol + seg],
                        in_=ytile[:],
                    )
    if legalize:
        _legalize_waits(nc)
        _audit_waits(nc)
    _NC_CACHE[key] = nc
    return nc


_ES_COUNTER = [0]


def _legalize_waits(nc):
    """walrus (this CoreV3 pin) accepts one sync wait per instruction (two on
    EventSemaphore); Tile sometimes emits more. Two fixes, in order:
      1. drop same-engine self-waits (a serial engine already executes its
         own stream in order, so a wait on its own proc lane is redundant);
      2. hoist still-excess waits onto EventSemaphore instructions inserted
         right before the offender on the same engine queue.
    """
    for b in nc.m.functions[0].blocks:
        il = b.instructions
        idx = 0
        while idx < len(il):
            i = il[idx]
            si = i.sync_info
            cap = 2 if i.opcode == "EventSemaphore" else 1
            if si is None or len(si.on_wait) <= cap:
                idx += 1
                continue
            eng = str(i.engine).split(".")[-1]
            keeps = []
            for w in si.on_wait:
                rest = None
                if w.ant_name.startswith(f"{eng}_sequencer_"):
                    rest = w.ant_name[len(eng) + 11 :]
                elif w.ant_name.startswith(f"{eng}_"):
                    rest = w.ant_name[len(eng) + 1 :]
                if rest is not None and rest.isdigit():
                    continue  # self-wait: implied by program order
                keeps.append(w)
            hoist, tail = keeps[:-cap], keeps[-cap:]
            while hoist:
                chunk, hoist = hoist[:2], hoist[2:]
                _ES_COUNTER[0] += 1
                es = mybir.InstEventSemaphore(
                    name=f"legalize-es-{_ES_COUNTER[0]}", ins=[], outs=[]
                )
                es.engine = i.engine
                es.sync_info = mybir.SyncInfo(on_wait=chunk, on_update=[])
                il.insert(idx, es)
                idx += 1
            i.sync_info = mybir.SyncInfo(on_wait=tail, on_update=list(si.on_update))
            idx += 1


def _audit_waits(nc):
    """walrus (CoreV3) accepts at most one sync wait per instruction
    (two on EventSemaphore). Fail at build time instead of compile time."""
    bad = []
    for b in nc.m.functions[0].blocks:
        for i in b.instructions:
            si = i.sync_info
            if si is None:
                continue
            cap = 2 if i.opcode == "EventSemaphore" else 1
            if len(si.on_wait) > cap:
                bad.append((i.name, i.opcode, len(si.on_wait)))
    if bad:
        raise AssertionError(f"instructions with too many waits: {bad[:10]}")


def _in_maps(x, weight):
    x = np.ascontiguousarray(np.asarray(x, dtype=np.float32))
    weight = np.ascontiguousarray(np.asarray(weight, dtype=np.float32))
    ones = np.ones((1, 128), dtype=np.float32)
    return [
        {"x": x[i * B_LOC : (i + 1) * B_LOC], "weight": weight, "onesr": ones}
        for i in range(N_CORES)
    ]


def run(x, weight, **spmd_kwargs):
    nc = _build()
    res = run_bass_kernel_spmd(
        nc, _in_maps(x, weight), core_ids=list(range(N_CORES)), **spmd_kwargs
    )
    out = np.concatenate([r["y"] for r in res.results], axis=0)
    return out, res


def kernel(x, weight):
    out, _ = run(x, weight)
    return out
